# revision 1
# baseline (speedup 1.0000x reference)
"""Bidirectional 2-layer ConvLSTM (3x3 grid) + FC head, Trainium2 Bass kernel.

Sharding: data-parallel over batch. B=64 across 8 cores -> 8 batches/core.
Weights replicated; no inter-core communication.

Per-core pipeline (single NEFF):
  A) transpose x to channel-major (PE transpose), layer-0 input projections
     for both directions as bf16 tap-accumulated matmuls -> DRAM zx0
  B) layer-0 recurrence, fwd+bwd chains interleaved per step; conv(h) via
     9-tap matmuls from a zero-padded h tile; gates on ACT/DVE
  C) layer-1 input projections from h0 = hf0 + hb0 -> DRAM zx1
  D) layer-1 recurrence (same as B)
  E) FC head on TensorE -> out [7, T*BL]
"""

import numpy as np
import ml_dtypes

import concourse.bass as bass
import concourse.mybir as mybir
from concourse.tile import TileContext
from concourse.masks import make_identity

BF16 = mybir.dt.bfloat16
F32 = mybir.dt.float32

B_FULL, T_FULL, C_IN, H, NCLS = 64, 128, 256, 128, 7
NCORES = 8
BL = B_FULL // NCORES  # local batch = 8
CLIPPED = True  # clipped-tap matmuls (multi-dim PSUM out APs, HW-validated)

# taps ordered center-first so the first matmul of each accumulation group
# covers every output column (has_written semantics)
TAPS = [(1, 1)] + [(dy, dx) for dy in range(3) for dx in range(3) if (dy, dx) != (1, 1)]


def _clip(d):
    # output-pixel range [p0, p0+n) and source range [s0, s0+n) for tap offset d
    if d == 0:
        return 1, 0, 2
    if d == 1:
        return 0, 0, 3
    return 0, 1, 2


def _patch_tile_drain():
    """This walrus rejects >1 sync wait on a Drain: keep the first wait on the
    drain and move the rest onto single-wait NOPs executed just before it."""
    from bass_rust import ScopedClock

    if getattr(TileContext, "_drain_patched", False):
        return

    def _drain_and_barrier(self, tick_clock, wait_clock):
        nc = self.nc
        drain_inst = nc.sync.drain()
        wait_clock.add_sem_waits(
            drain_inst.ins, ScopedClock({None: tick_clock.global_clock})
        )
        si = drain_inst.ins.sync_info
        waits = list(si.on_wait)
        if len(waits) > 1:
            while len(si.on_wait) > 1:
                si.on_wait.pop()
            for w in waits[1:]:
                nop = nc.sync.nop()
                nop.ins.sync_info = mybir.SyncInfo(on_wait=[w], on_update=[])
        nc.all_engine_barrier()
        assert self.sems is not None
        popped = nc._tile_sem_poison_stack.pop()
        assert popped is self._sem_poison
        nc.clear_and_free_semaphores(list(self.sems.allocated().values()))
        nc.all_engine_barrier()

    TileContext._drain_and_barrier = _drain_and_barrier
    TileContext._drain_patched = True


def _fix_multi_waits(raw):
    """This walrus accepts at most 1 sync wait per instruction (2 for
    EventSemaphore). Hoist excess waits onto single-wait EventSemaphore
    carriers inserted just before the instruction on the same engine."""
    import json

    d = json.loads(raw)
    nid = 0
    for fn in d["functions"]:
        for blk in fn["blocks"]:
            out = []
            for inst in blk["instructions"]:
                si = inst.get("sync_info")
                ow = (si or {}).get("on_wait") or []
                cap = 2 if inst.get("opcode") == "EventSemaphore" else 1
                if len(ow) > cap:
                    for w in ow[cap:]:
                        nid += 1
                        out.append({
                            "debug": inst.get("debug", 0),
                            "engine": inst["engine"],
                            "ins": [],
                            "name": f"I-xwait-{nid}",
                            "opcode": "EventSemaphore",
                            "outs": [],
                            "sync_info": {"on_update": [], "on_wait": [w]},
                        })
                    si["on_wait"] = ow[:cap]
                out.append(inst)
            blk["instructions"] = out
    return json.dumps(d).encode()


def build_program(T=T_FULL, phases="ABCDE"):
    """Build the per-core Bass program. Returns nc."""
    _patch_tile_drain()
    G = T * BL  # column groups, g = t*BL + b
    GT = 128 if G % 128 == 0 else G  # groups per transpose tile
    assert G % GT == 0 and GT % 32 == 0
    n_gt = G // GT
    n_sub = GT // 32

    nc = bass.Bass()

    # ---- I/O ----
    x = nc.dram_tensor("x", [BL, T, C_IN, 3, 3], F32, kind="ExternalInput")
    wx0 = {}
    wh0 = {}
    wx1 = {}
    wh1 = {}
    bias_in = {}
    for d in ("f", "b"):
        wx0[d] = nc.dram_tensor(f"wx0{d}", [128, 2, 9, 512], BF16, kind="ExternalInput")
        wh0[d] = nc.dram_tensor(f"wh0{d}", [128, 9, 512], BF16, kind="ExternalInput")
        wx1[d] = nc.dram_tensor(f"wx1{d}", [128, 9, 512], BF16, kind="ExternalInput")
        wh1[d] = nc.dram_tensor(f"wh1{d}", [128, 9, 512], BF16, kind="ExternalInput")
        bias_in[f"0{d}"] = nc.dram_tensor(f"bias0{d}", [128, 4], F32, kind="ExternalInput")
        bias_in[f"1{d}"] = nc.dram_tensor(f"bias1{d}", [128, 4], F32, kind="ExternalInput")
    fcw = nc.dram_tensor("fcw", [128, 9, NCLS], BF16, kind="ExternalInput")
    fcb = nc.dram_tensor("fcb", [NCLS, 1], F32, kind="ExternalInput")
    out = nc.dram_tensor("out", [NCLS, G], F32, kind="ExternalOutput")

    # ---- DRAM scratch ----
    zx0 = {d: nc.dram_tensor(f"zx0{d}", [4, 128, G, 9], F32) for d in ("f", "b")}
    zx1 = {d: nc.dram_tensor(f"zx1{d}", [4, 128, G, 9], F32) for d in ("f", "b")}
    h0d = {d: nc.dram_tensor(f"h0{d}", [128, G, 9], F32) for d in ("f", "b")}
    h1d = {d: nc.dram_tensor(f"h1{d}", [128, G, 9], F32) for d in ("f", "b")}

    with TileContext(nc) as tc:
        with tc.tile_pool(name="persist", bufs=1) as pp:
            # persistent weights in SBUF
            wx0_sb = {d: pp.tile([128, 2, 9, 512], BF16, name=f"wx0{d}", tag=f"wx0{d}") for d in ("f", "b")}
            wh0_sb = {d: pp.tile([128, 9, 512], BF16, name=f"wh0{d}", tag=f"wh0{d}") for d in ("f", "b")}
            wx1_sb = {d: pp.tile([128, 9, 512], BF16, name=f"wx1{d}", tag=f"wx1{d}") for d in ("f", "b")}
            wh1_sb = {d: pp.tile([128, 9, 512], BF16, name=f"wh1{d}", tag=f"wh1{d}") for d in ("f", "b")}
            bias_sb = {}
            for d in ("f", "b"):
                nc.sync.dma_start(out=wx0_sb[d][:], in_=wx0[d][:])
                nc.sync.dma_start(out=wh0_sb[d][:], in_=wh0[d][:])
                nc.sync.dma_start(out=wx1_sb[d][:], in_=wx1[d][:])
                nc.sync.dma_start(out=wh1_sb[d][:], in_=wh1[d][:])
                for l in ("0", "1"):
                    bias_sb[l + d] = pp.tile([128, 4], F32, name=f"bias{l}{d}", tag=f"bias{l}{d}")
                    nc.sync.dma_start(out=bias_sb[l + d][:], in_=bias_in[l + d][:])
            fcw_sb = pp.tile([128, 9, NCLS], BF16, tag="fcw")
            nc.sync.dma_start(out=fcw_sb[:], in_=fcw[:])
            fcb_sb = pp.tile([NCLS, 1], F32, tag="fcb")
            nc.sync.dma_start(out=fcb_sb[:], in_=fcb[:])
            ident = pp.tile([128, 128], F32, tag="ident")
            make_identity(nc, ident[:])

            # ================= Phase A: layer-0 x projections =================
            # xpad: channel-major zero-padded x, double-buffered over g-tiles.
            # Borders memset once; interiors fully rewritten each g-tile.
            xpad = [
                pp.tile([128, 2, GT, 5, 5], BF16, name=f"xpad{par}", tag=f"xpad{par}")
                for par in range(2)
            ]
            nc.gpsimd.memset(xpad[0][:], 0.0)
            nc.gpsimd.memset(xpad[1][:], 0.0)
            x_gv = x[:].rearrange("b t c y x -> t b (c y x)")  # (T, BL, 2304)
            with (
                tc.tile_pool(name="a_xg", bufs=2) as xgp,
                tc.tile_pool(name="a_tp", bufs=2, space="PSUM") as tpp,
                tc.tile_pool(name="a_zp", bufs=5, space="PSUM") as zpp,
                tc.tile_pool(name="a_zs", bufs=3) as zsp,
            ):
                for gt in range(n_gt if "A" in phases else 0):
                    t0 = gt * (GT // BL)
                    nt = GT // BL
                    xg = xgp.tile([GT, C_IN * 9], F32, tag="xg")
                    # one DMA per time-step: SBUF partition dim must stay a
                    # single dim (split-partition DMA misbehaves on HW)
                    for ts in range(nt):
                        nc.sync.dma_start(
                            out=xg[ts * BL : (ts + 1) * BL, :],
                            in_=x_gv[t0 + ts],
                        )
                    xgv = xg[:].rearrange("g (c y x) -> g c y x", y=3, x=3)
                    xp = xpad[gt % 2]
                    for cb in range(2):
                        for y in range(3):
                            for xx in range(3):
                                tp = tpp.tile([128, GT], F32, tag="tp")
                                nc.tensor.transpose(
                                    tp[:], xgv[:, cb * 128 : (cb + 1) * 128, y, xx],
                                    ident[:GT, :GT],
                                )
                                nc.vector.tensor_copy(
                                    xp[:, cb, :, 1 + y, 1 + xx], tp[:]
                                )
                    for sub in range(n_sub):
                        g0 = sub * 32
                        for d in ("f", "b"):
                            zs = zsp.tile([128, 4, 32, 9], F32, tag="zs")
                            for cb_o in range(4):
                                zp = zpp.tile([128, 32, 3, 3], F32, tag="zp")
                                zpf = zp[:].rearrange("p g y x -> p (g y x)")
                                k = 0
                                for dy, dx in TAPS:
                                    py, sy, ny = _clip(dy)
                                    px, sx, nx2 = _clip(dx)
                                    for cb_i in range(2):
                                        w_ap = wx0_sb[d][
                                            :, cb_i, dy * 3 + dx,
                                            cb_o * 128 : (cb_o + 1) * 128,
                                        ]
                                        if CLIPPED and not (ny == 3 and nx2 == 3):
                                            o_ap = zp[:, :, py : py + ny, px : px + nx2]
                                            r_ap = xp[
                                                :, cb_i, g0 : g0 + 32,
                                                1 + sy : 1 + sy + ny,
                                                1 + sx : 1 + sx + nx2,
                                            ]
                                        else:
                                            o_ap = zpf
                                            r_ap = xp[
                                                :, cb_i, g0 : g0 + 32,
                                                dy : dy + 3, dx : dx + 3,
                                            ]
                                        nc.tensor.matmul(
                                            o_ap, w_ap, r_ap,
                                            start=(k == 0),
                                            stop=(k == 17),
                                        )
                                        k += 1
                                nc.vector.tensor_copy(
                                    zs[:, cb_o],
                                    zp[:].rearrange("p g y x -> p g (y x)"),
                                )
                            nc.sync.dma_start(
                                out=zx0[d][
                                    :, :, gt * GT + g0 : gt * GT + g0 + 32, :
                                ].rearrange("cb p g yx -> p cb g yx"),
                                in_=zs[:],
                            )

            # ================= Phase B: layer-0 recurrence =================
            _recurrence(nc, tc, T if "B" in phases else 0, wh0_sb,
                        {d: bias_sb["0" + d] for d in ("f", "b")}, zx0, h0d, "l0")

            # ================= Phase C: layer-1 x projections =================
            h0pad = [
                pp.tile([128, GT, 5, 5], BF16, name=f"h0pad{par}", tag=f"h0pad{par}")
                for par in range(2)
            ]
            nc.gpsimd.memset(h0pad[0][:], 0.0)
            nc.gpsimd.memset(h0pad[1][:], 0.0)
            with (
                tc.tile_pool(name="c_h", bufs=3) as chp,
                tc.tile_pool(name="c_zp", bufs=5, space="PSUM") as zpp,
                tc.tile_pool(name="c_zs", bufs=3) as zsp,
            ):
                for gt in range(n_gt if "C" in phases else 0):
                    ga = gt * GT
                    hf_t = chp.tile([128, GT, 3, 3], F32, tag="hf")
                    hb_t = chp.tile([128, GT, 3, 3], F32, tag="hb")
                    nc.sync.dma_start(
                        out=hf_t[:].rearrange("p g y x -> p g (y x)"),
                        in_=h0d["f"][:, ga : ga + GT, :],
                    )
                    nc.sync.dma_start(
                        out=hb_t[:].rearrange("p g y x -> p g (y x)"),
                        in_=h0d["b"][:, ga : ga + GT, :],
                    )
                    hp = h0pad[gt % 2]
                    nc.vector.tensor_add(hp[:, :, 1:4, 1:4], hf_t[:], hb_t[:])
                    for sub in range(n_sub):
                        g0 = sub * 32
                        for d in ("f", "b"):
                            zs = zsp.tile([128, 4, 32, 9], F32, tag="zs")
                            for cb_o in range(4):
                                zp = zpp.tile([128, 32, 3, 3], F32, tag="zp")
                                zpf = zp[:].rearrange("p g y x -> p (g y x)")
                                for k, (dy, dx) in enumerate(TAPS):
                                    py, sy, ny = _clip(dy)
                                    px, sx, nx2 = _clip(dx)
                                    w_ap = wx1_sb[d][
                                        :, dy * 3 + dx,
                                        cb_o * 128 : (cb_o + 1) * 128,
                                    ]
                                    if CLIPPED and not (ny == 3 and nx2 == 3):
                                        o_ap = zp[:, :, py : py + ny, px : px + nx2]
                                        r_ap = hp[
                                            :, g0 : g0 + 32,
                                            1 + sy : 1 + sy + ny,
                                            1 + sx : 1 + sx + nx2,
                                        ]
                                    else:
                                        o_ap = zpf
                                        r_ap = hp[
                                            :, g0 : g0 + 32, dy : dy + 3, dx : dx + 3
                                        ]
                                    nc.tensor.matmul(
                                        o_ap, w_ap, r_ap,
                                        start=(k == 0),
                                        stop=(k == 8),
                                    )
                                nc.vector.tensor_copy(
                                    zs[:, cb_o],
                                    zp[:].rearrange("p g y x -> p g (y x)"),
                                )
                            nc.sync.dma_start(
                                out=zx1[d][
                                    :, :, ga + g0 : ga + g0 + 32, :
                                ].rearrange("cb p g yx -> p cb g yx"),
                                in_=zs[:],
                            )

            # ================= Phase D: layer-1 recurrence =================
            _recurrence(nc, tc, T if "D" in phases else 0, wh1_sb,
                        {d: bias_sb["1" + d] for d in ("f", "b")}, zx1, h1d, "l1")

            # ================= Phase E: FC head =================
            with (
                tc.tile_pool(name="e_h", bufs=3) as ehp,
                tc.tile_pool(name="e_ps", bufs=2, space="PSUM") as epp,
                tc.tile_pool(name="e_o", bufs=2) as eop,
            ):
                EC = min(128, G)  # groups per chunk
                assert G % EC == 0
                for gc in range(G // EC if "E" in phases else 0):
                    ga = gc * EC
                    hf_t = ehp.tile([128, EC, 9], F32, tag="ehf")
                    hb_t = ehp.tile([128, EC, 9], F32, tag="ehb")
                    h1s = ehp.tile([128, EC, 9], BF16, tag="eh1s")
                    nc.sync.dma_start(out=hf_t[:], in_=h1d["f"][:, ga : ga + EC, :])
                    nc.sync.dma_start(out=hb_t[:], in_=h1d["b"][:, ga : ga + EC, :])
                    nc.vector.tensor_add(h1s[:], hf_t[:], hb_t[:])
                    ps = epp.tile([NCLS, EC], F32, tag="eps")
                    for yx in range(9):
                        nc.tensor.matmul(
                            ps[:],
                            fcw_sb[:, yx, :],
                            h1s[:, :, yx],
                            start=(yx == 0),
                            stop=(yx == 8),
                        )
                    ot = eop.tile([NCLS, EC], F32, tag="eo")
                    nc.vector.tensor_scalar_add(ot[:], ps[:], fcb_sb[:, 0:1])
                    nc.sync.dma_start(out=out[:, ga : ga + EC], in_=ot[:])

    _orig_to_json = nc.to_json_bytes
    nc.to_json_bytes = lambda: _fix_multi_waits(_orig_to_json())
    return nc


def _recurrence(nc, tc, T, wh_sb, bias, zx, hout, name):
    """One bidirectional ConvLSTM recurrence. wh_sb/bias/zx/hout keyed by dir."""
    SIG = mybir.ActivationFunctionType.Sigmoid
    TANH = mybir.ActivationFunctionType.Tanh
    with (
        tc.tile_pool(name=f"{name}_st", bufs=1) as stp,
        tc.tile_pool(name=f"{name}_zx", bufs=6) as zxp,
        tc.tile_pool(name=f"{name}_ps", bufs=4, space="PSUM") as psp,
        tc.tile_pool(name=f"{name}_g", bufs=3) as gp,
    ):
        hpad = {}
        cst = {}
        for d in ("f", "b"):
            hpad[d] = [stp.tile([128, BL, 5, 5], BF16, name=f"{name}hp{d}{par}", tag=f"{name}hp{d}{par}") for par in range(2)]
            nc.gpsimd.memset(hpad[d][0][:], 0.0)
            nc.gpsimd.memset(hpad[d][1][:], 0.0)
            cst[d] = stp.tile([128, BL * 9], F32, name=f"{name}c{d}", tag=f"{name}c{d}")
            nc.gpsimd.memset(cst[d][:], 0.0)
        for s in range(T):
            for d in ("f", "b"):
                t = s if d == "f" else T - 1 - s
                hp_r = hpad[d][s % 2]
                hp_w = hpad[d][(s + 1) % 2]
                zxt = zxp.tile([128, 4, BL * 9], F32, name=f"zxt{d}", tag=f"zx{d}")
                nc.sync.dma_start(
                    out=zxt[:],
                    in_=zx[d][:, :, t * BL : (t + 1) * BL, :].rearrange(
                        "cb p b yx -> p cb (b yx)"
                    ),
                )
                zp = psp.tile([128, 4, BL * 9], F32, name=f"zp{d}", tag=f"zp{d}")
                for cb in range(4):
                    for k, (dy, dx) in enumerate(TAPS):
                        nc.tensor.matmul(
                            zp[:, cb],
                            wh_sb[d][:, dy * 3 + dx, cb * 128 : (cb + 1) * 128],
                            hp_r[:, :, dy : dy + 3, dx : dx + 3],
                            start=(k == 0),
                            stop=(k == 8),
                        )
                z = gp.tile([128, 4, BL * 9], F32, name=f"z{d}", tag=f"z{d}")
                nc.vector.tensor_add(z[:], zp[:], zxt[:])
                si = gp.tile([128, BL * 9], F32, name=f"si{d}", tag=f"si{d}")
                sf = gp.tile([128, BL * 9], F32, name=f"sf{d}", tag=f"sf{d}")
                so = gp.tile([128, BL * 9], F32, name=f"so{d}", tag=f"so{d}")
                tg = gp.tile([128, BL * 9], F32, name=f"tg{d}", tag=f"tg{d}")
                nc.scalar.activation(si[:], z[:, 0], SIG, bias=bias[d][:, 0:1])
                nc.scalar.activation(sf[:], z[:, 1], SIG, bias=bias[d][:, 1:2])
                nc.scalar.activation(so[:], z[:, 2], SIG, bias=bias[d][:, 2:3])
                nc.scalar.activation(tg[:], z[:, 3], TANH, bias=bias[d][:, 3:4])
                ig = gp.tile([128, BL * 9], F32, name=f"ig{d}", tag=f"ig{d}")
                nc.vector.tensor_mul(ig[:], si[:], tg[:])
                cf = gp.tile([128, BL * 9], F32, name=f"cf{d}", tag=f"cf{d}")
                nc.vector.tensor_mul(cf[:], sf[:], cst[d][:])
                nc.vector.tensor_add(cst[d][:], ig[:], cf[:])
                tcell = gp.tile([128, BL * 9], F32, name=f"tcl{d}", tag=f"tc{d}")
                nc.scalar.activation(tcell[:], cst[d][:], TANH)
                h = gp.tile([128, BL * 9], F32, name=f"h{d}", tag=f"h{d}")
                nc.vector.tensor_mul(h[:], so[:], tcell[:])
                nc.vector.tensor_copy(
                    hp_w[:, :, 1:4, 1:4],
                    h[:].rearrange("p (b y x) -> p b y x", y=3, x=3),
                )
                nc.sync.dma_start(
                    out=hout[d][:, t * BL : (t + 1) * BL, :].rearrange(
                        "p b yx -> p (b yx)"
                    ),
                    in_=h[:],
                )


# ---------------- host side ----------------

def _prep_weights(w, b, cin):
    """w: (512, cin+128, 3, 3) -> (wx, wh) bf16 host arrays + bias (128,4) f32."""
    bf = ml_dtypes.bfloat16
    wx = w[:, :cin].reshape(512, cin, 9)            # (co, ci, tap)
    wx = wx.transpose(1, 2, 0)                      # (ci, tap, co)
    if cin == 256:
        wx = wx.reshape(2, 128, 9, 512).transpose(1, 0, 2, 3)  # (128, 2, 9, 512)
    wx = np.ascontiguousarray(wx).astype(bf)
    wh = w[:, cin:].reshape(512, 128, 9).transpose(1, 2, 0)    # (128, 9, 512)
    wh = np.ascontiguousarray(wh).astype(bf)
    bias = np.ascontiguousarray(b.reshape(4, 128).T).astype(np.float32)
    return wx, wh, bias


def make_inputs_core(core, x, w_f0, b_f0, w_b0, b_b0, w_f1, b_f1, w_b1, b_b1,
                     fc_w, fc_b):
    m = {"x": np.ascontiguousarray(x[core * BL : (core + 1) * BL])}
    for d, w, b in (("f", w_f0, b_f0), ("b", w_b0, b_b0)):
        wx, wh, bias = _prep_weights(np.asarray(w), np.asarray(b), 256)
        m[f"wx0{d}"], m[f"wh0{d}"], m[f"bias0{d}"] = wx, wh, bias
    for d, w, b in (("f", w_f1, b_f1), ("b", w_b1, b_b1)):
        wx, wh, bias = _prep_weights(np.asarray(w), np.asarray(b), 128)
        m[f"wx1{d}"], m[f"wh1{d}"], m[f"bias1{d}"] = wx, wh, bias
    fcw = np.asarray(fc_w).reshape(NCLS, 128, 9).transpose(1, 2, 0)  # (128, 9, 7)
    m["fcw"] = np.ascontiguousarray(fcw).astype(ml_dtypes.bfloat16)
    m["fcb"] = np.ascontiguousarray(np.asarray(fc_b).reshape(NCLS, 1)).astype(np.float32)
    return m


_nc_cache = {}


def kernel(**inputs):
    from concourse.bass_utils import run_bass_kernel_spmd

    if "nc" not in _nc_cache:
        _nc_cache["nc"] = build_program(T_FULL)
    nc = _nc_cache["nc"]
    x = np.asarray(inputs["x"], dtype=np.float32)
    in_maps = [make_inputs_core(c, x, inputs["w_f0"], inputs["b_f0"],
                                inputs["w_b0"], inputs["b_b0"],
                                inputs["w_f1"], inputs["b_f1"],
                                inputs["w_b1"], inputs["b_b1"],
                                inputs["fc_w"], inputs["fc_b"])
               for c in range(NCORES)]
    res = run_bass_kernel_spmd(nc, in_maps, core_ids=list(range(NCORES)))
    outs = []
    for c in range(NCORES):
        o = res.results[c]["out"]  # (7, G) with g = t*BL + b
        o = o.reshape(NCLS, T_FULL, BL).transpose(2, 1, 0)  # (BL, T, 7)
        outs.append(o)
    return np.ascontiguousarray(np.concatenate(outs, axis=0), dtype=np.float32)



# revision 27
# speedup vs baseline: 1.2844x; 1.2844x over previous
"""Bidirectional 2-layer ConvLSTM (3x3 grid) + FC head, Trainium2 Bass kernel.

Sharding: data-parallel over batch. B=64 across 8 cores -> 8 batches/core.
Weights replicated; no inter-core communication.

Single merged instruction stream per core. The recurrence (B/D) leaves
~1.1us PE bubbles per step (gate-chain latency exceeds the other
direction's matmul work), so the feed-forward phases are woven into those
bubbles instead of running as separate phases:

  prelude: x-projection chunks 0,7,1,6 (both time-order heads)
  B steps 0..47   (+) x-projection chunks 2,5,3,4 as filler
  B steps 80..127 (+) layer-1 projection chunks 3,4,2,5 (ready mid-B)
  C chunks 0,7; D steps 0..15 (+) C chunks 1,6; D tail (+) FC chunks

zx scratch lives in DRAM as per-chunk tensors (16 time-steps each) so
dependency tracking stays chunk-granular. zx is bf16 with biases folded
in; it is added into the conv PSUM accumulation via a bf16 identity
matmul, and gates read straight from PSUM. h stays in SBUF (bf16).
"""

import numpy as np
import ml_dtypes

import concourse.bass as bass
import concourse.mybir as mybir
from concourse.tile import TileContext
from concourse.masks import make_identity

BF16 = mybir.dt.bfloat16
F32 = mybir.dt.float32
FP8 = mybir.dt.float8e4
WSCALE = 64.0  # fp8 weight pre-scale: keeps |w*S| ~1, clear of e4m3 subnormals

B_FULL, T_FULL, C_IN, H, NCLS = 64, 128, 256, 128, 7
NCORES = 8
BL = B_FULL // NCORES  # local batch = 8
ZB = 4  # zx DMA time-step batching in the recurrence
CLIPPED = True  # clipped-tap matmuls (multi-dim PSUM out APs, HW-validated)

# taps ordered center-first so the first matmul of each accumulation group
# covers every output column (has_written semantics)
TAPS = [(1, 1)] + [(dy, dx) for dy in range(3) for dx in range(3) if (dy, dx) != (1, 1)]

SIG = mybir.ActivationFunctionType.Sigmoid
TANH = mybir.ActivationFunctionType.Tanh


def _clip(d):
    # output-pixel range [p0, p0+n) and source range [s0, s0+n) for tap offset d
    if d == 0:
        return 1, 0, 2
    if d == 1:
        return 0, 0, 3
    return 0, 1, 2


def _patch_tile_drain():
    """This walrus rejects >1 sync wait on a Drain: keep the first wait on the
    drain and move the rest onto single-wait NOPs executed just before it."""
    from bass_rust import ScopedClock

    if getattr(TileContext, "_drain_patched", False):
        return

    def _drain_and_barrier(self, tick_clock, wait_clock):
        nc = self.nc
        drain_inst = nc.sync.drain()
        wait_clock.add_sem_waits(
            drain_inst.ins, ScopedClock({None: tick_clock.global_clock})
        )
        si = drain_inst.ins.sync_info
        waits = list(si.on_wait)
        if len(waits) > 1:
            while len(si.on_wait) > 1:
                si.on_wait.pop()
            for w in waits[1:]:
                nop = nc.sync.nop()
                nop.ins.sync_info = mybir.SyncInfo(on_wait=[w], on_update=[])
        nc.all_engine_barrier()
        assert self.sems is not None
        popped = nc._tile_sem_poison_stack.pop()
        assert popped is self._sem_poison
        nc.clear_and_free_semaphores(list(self.sems.allocated().values()))
        nc.all_engine_barrier()

    TileContext._drain_and_barrier = _drain_and_barrier
    TileContext._drain_patched = True


def _fix_multi_waits(raw):
    """This walrus accepts at most 1 sync wait per instruction (2 for
    EventSemaphore). Hoist excess waits onto single-wait EventSemaphore
    carriers inserted just before the instruction on the same engine."""
    import json

    d = json.loads(raw)
    nid = 0
    for fn in d["functions"]:
        for blk in fn["blocks"]:
            out = []
            for inst in blk["instructions"]:
                si = inst.get("sync_info")
                ow = (si or {}).get("on_wait") or []
                cap = 2 if inst.get("opcode") == "EventSemaphore" else 1
                if len(ow) > cap:
                    for w in ow[cap:]:
                        nid += 1
                        out.append({
                            "debug": inst.get("debug", 0),
                            "engine": inst["engine"],
                            "ins": [],
                            "name": f"I-xwait-{nid}",
                            "opcode": "EventSemaphore",
                            "outs": [],
                            "sync_info": {"on_update": [], "on_wait": [w]},
                        })
                    si["on_wait"] = ow[:cap]
                out.append(inst)
            blk["instructions"] = out
    return json.dumps(d).encode()


class _Recurrence:
    """One bidirectional ConvLSTM layer, emitted one step at a time so
    feed-forward filler work can be woven between steps."""

    def __init__(self, nc, stp, zxp, psp, gp, wh_sb, zxc, h_sb, identb, name,
                 T, chunk_t):
        self.nc = nc
        self.zxp = zxp
        self.psp = psp
        self.gp = gp
        self.wh_sb = wh_sb
        self.zxc = zxc  # per-chunk DRAM zx tensors, keyed by dir
        self.h_sb = h_sb
        self.identb = identb
        self.name = name
        self.T = T
        self.chunk_t = chunk_t
        self.zxt = {}
        self.hpad = {}
        self.cst = {}
        for d in ("f", "b"):
            self.hpad[d] = [
                stp.tile([128, BL, 5, 5], BF16, name=f"{name}hp{d}{p}", tag=f"{name}hp{d}{p}")
                for p in range(2)
            ]
            nc.gpsimd.memset(self.hpad[d][0][:], 0.0)
            nc.gpsimd.memset(self.hpad[d][1][:], 0.0)
            self.cst[d] = stp.tile([128, BL * 9], F32, name=f"{name}c{d}", tag=f"{name}c{d}")
            nc.gpsimd.memset(self.cst[d][:], 0.0)

    def step(self, s):
        nc = self.nc
        T = self.T
        tt = {"f": s, "b": T - 1 - s}
        zp = {}
        sfio = {}
        tg = {}
        # PE: both dirs' matmuls first, so the engine has a full step of
        # runway while the other dir's gate chain drains
        for d in ("f", "b"):
            t = tt[d]
            hp_r = self.hpad[d][s % 2]
            if s % ZB == 0:
                tz = t if d == "f" else t - (ZB - 1)
                ck = tz // self.chunk_t
                tl = tz % self.chunk_t
                zt = self.zxp.tile([128, 4, ZB, BL * 9], BF16,
                                   name=f"{self.name}zx{d}", tag=f"{self.name}zx{d}")
                nc.sync.dma_start(
                    out=zt[:],
                    in_=self.zxc[d][ck][:, :, tl * BL : (tl + ZB) * BL, :].rearrange(
                        "cb p (zb b) yx -> p cb zb (b yx)", zb=ZB
                    ),
                )
                self.zxt[d] = zt
            zi = s % ZB if d == "f" else ZB - 1 - s % ZB
            zp[d] = self.psp.tile([128, 4, BL * 9], F32,
                                  name=f"{self.name}zp{d}", tag=f"{self.name}zp{d}")
            # g-gate block (cb 3) first: its tanh runs on ACT while PE works
            # through the i/f/o blocks, taking it off the gate critical path
            for cb in (3, 0, 1, 2):
                nc.tensor.matmul(
                    zp[d][:, cb], self.identb[:], self.zxt[d][:, cb, zi],
                    start=True, stop=False,
                )
                for k, (dy, dx) in enumerate(TAPS):
                    nc.tensor.matmul(
                        zp[d][:, cb],
                        self.wh_sb[d][:, dy * 3 + dx, cb * 128 : (cb + 1) * 128],
                        hp_r[:, :, dy : dy + 3, dx : dx + 3],
                        start=False, stop=(k == 8),
                    )
                if cb == 3:
                    tg[d] = self.gp.tile([128, BL * 9], F32,
                                         name=f"{self.name}tg{d}", tag=f"{self.name}tg{d}")
                    nc.scalar.activation(tg[d][:], zp[d][:, 3], TANH)
        # gates: one sigmoid over i,f,o; reads straight from PSUM
        for d in ("f", "b"):
            sfio[d] = self.gp.tile([128, 3, BL * 9], F32,
                                   name=f"{self.name}sfio{d}", tag=f"{self.name}sfio{d}")
            nc.scalar.activation(sfio[d][:], zp[d][:, 0:3], SIG)
        tcell = {}
        for d in ("f", "b"):
            ig = self.gp.tile([128, BL * 9], F32, name=f"{self.name}ig{d}", tag=f"{self.name}ig{d}")
            cf = self.gp.tile([128, BL * 9], F32, name=f"{self.name}cf{d}", tag=f"{self.name}cf{d}")
            nc.vector.tensor_mul(ig[:], sfio[d][:, 0], tg[d][:])
            nc.vector.tensor_mul(cf[:], sfio[d][:, 1], self.cst[d][:])
            nc.vector.tensor_add(self.cst[d][:], ig[:], cf[:])
            tcell[d] = self.gp.tile([128, BL * 9], F32, name=f"{self.name}tc{d}", tag=f"{self.name}tc{d}")
            nc.scalar.activation(tcell[d][:], self.cst[d][:], TANH)
        for d in ("f", "b"):
            # h = so * tanh(c): straight into the padded tile (critical path
            # to the next step's conv, on DVE) and into the SBUF h buffer on
            # the otherwise-idle GPSIMD engine (off the critical path)
            hp_w = self.hpad[d][(s + 1) % 2]
            nc.vector.tensor_mul(
                hp_w[:, :, 1:4, 1:4],
                sfio[d][:, 2].rearrange("p (b y x) -> p b y x", y=3, x=3),
                tcell[d][:].rearrange("p (b y x) -> p b y x", y=3, x=3),
            )
            nc.gpsimd.tensor_mul(
                self.h_sb[d][:, tt[d] * BL : (tt[d] + 1) * BL, :],
                sfio[d][:, 2].rearrange("p (b yx) -> p b yx", b=BL),
                tcell[d][:].rearrange("p (b yx) -> p b yx", b=BL),
            )


def build_program(T=T_FULL, phases="ABCDE"):
    """Build the per-core Bass program (merged stream). Returns nc."""
    _patch_tile_drain()
    G = T * BL
    GT = 128 if G % 128 == 0 else G  # groups per chunk
    assert G % GT == 0 and GT % 32 == 0
    n_gt = G // GT
    n_sub = GT // 32
    chunk_t = GT // BL  # time-steps per chunk (16)
    assert n_gt == 8 and chunk_t % ZB == 0

    nc = bass.Bass()

    # ---- I/O ----
    x = nc.dram_tensor("x", [BL, T, C_IN, 3, 3], F32, kind="ExternalInput")
    wx0 = {}
    wh0 = {}
    wx1 = {}
    wh1 = {}
    bias_in = {}
    for d in ("f", "b"):
        wx0[d] = nc.dram_tensor(f"wx0{d}", [128, 2, 9, 512], BF16, kind="ExternalInput")
        wh0[d] = nc.dram_tensor(f"wh0{d}", [128, 9, 512], BF16, kind="ExternalInput")
        wx1[d] = nc.dram_tensor(f"wx1{d}", [128, 9, 512], BF16, kind="ExternalInput")
        wh1[d] = nc.dram_tensor(f"wh1{d}", [128, 9, 512], BF16, kind="ExternalInput")
        bias_in[f"0{d}"] = nc.dram_tensor(f"bias0{d}", [128, 4], F32, kind="ExternalInput")
        bias_in[f"1{d}"] = nc.dram_tensor(f"bias1{d}", [128, 4], F32, kind="ExternalInput")
    fcw = nc.dram_tensor("fcw", [128, 9, NCLS], BF16, kind="ExternalInput")
    fcb = nc.dram_tensor("fcb", [NCLS, 1], F32, kind="ExternalInput")
    out = nc.dram_tensor("out", [NCLS, G], F32, kind="ExternalOutput")

    # ---- DRAM scratch: per-chunk zx tensors (bf16, biases folded in) ----
    zx0c = {d: [nc.dram_tensor(f"zx0{d}{k}", [4, 128, GT, 9], BF16) for k in range(n_gt)]
            for d in ("f", "b")}
    zx1c = {d: [nc.dram_tensor(f"zx1{d}{k}", [4, 128, GT, 9], BF16) for k in range(n_gt)]
            for d in ("f", "b")}

    with TileContext(nc) as tc:
        with tc.tile_pool(name="persist", bufs=1) as pp:
            wh0_sb = {d: pp.tile([128, 9, 512], BF16, name=f"wh0{d}", tag=f"wh0{d}") for d in ("f", "b")}
            wx1_sb = {d: pp.tile([128, 9, 512], BF16, name=f"wx1{d}", tag=f"wx1{d}") for d in ("f", "b")}
            wh1_sb = {d: pp.tile([128, 9, 512], BF16, name=f"wh1{d}", tag=f"wh1{d}") for d in ("f", "b")}
            bias_sb = {}
            big_dmas = []  # bulk weight loads, issued after the first x loads
            for d in ("f", "b"):
                big_dmas += [(wh0_sb[d], wh0[d]), (wx1_sb[d], wx1[d]),
                             (wh1_sb[d], wh1[d])]
                for l in ("0", "1"):
                    bias_sb[l + d] = pp.tile([128, 4], F32, name=f"bias{l}{d}", tag=f"bias{l}{d}")
                    nc.sync.dma_start(out=bias_sb[l + d][:], in_=bias_in[l + d][:])
            fcw_sb = pp.tile([128, 9, NCLS], BF16, tag="fcw")
            big_dmas.append((fcw_sb, fcw))
            fcb_sb = pp.tile([NCLS, 1], F32, tag="fcb")
            nc.sync.dma_start(out=fcb_sb[:], in_=fcb[:])
            ident = pp.tile([128, 128], F32, tag="ident")
            make_identity(nc, ident[:])
            identb = pp.tile([128, 128], BF16, tag="identb")
            make_identity(nc, identb[:])

            x_gv = x[:].rearrange("b t c y x -> t b (c y x)")  # (T, BL, 2304)

            def proj_mm_unit(xp_ap, n_cbi, w_of, zxt_d, bias_ap, g0):
                """One (sub, dir) projection unit: 4 gate blocks x taps
                matmuls + bias-fold copies (bf16) + one DMA store."""
                zs = zsp.tile([128, 4, 32, 9], BF16, tag="zs")
                for cb_o in range(4):
                    zp = zpp.tile([128, 32, 3, 3], F32, tag="zp")
                    zpf = zp[:].rearrange("p g y x -> p (g y x)")
                    k = 0
                    for dy, dx in TAPS:
                        py, sy, ny = _clip(dy)
                        px, sx, nx2 = _clip(dx)
                        clipped = CLIPPED and not (ny == 3 and nx2 == 3)
                        for cb_i in range(n_cbi):
                            w_ap = w_of(cb_i, dy * 3 + dx, cb_o)
                            if clipped:
                                o_ap = zp[:, :, py : py + ny, px : px + nx2]
                                r_ap = xp_ap(cb_i, g0, 1 + sy, ny, 1 + sx, nx2)
                            else:
                                o_ap = zpf
                                r_ap = xp_ap(cb_i, g0, dy, 3, dx, 3)
                            nc.tensor.matmul(
                                o_ap, w_ap, r_ap,
                                start=(k == 0), stop=(k == 9 * n_cbi - 1),
                            )
                            k += 1
                    nc.vector.tensor_scalar_add(
                        zs[:, cb_o],
                        zp[:].rearrange("p g y x -> p g (y x)"),
                        bias_ap[:, cb_o : cb_o + 1],
                    )
                nc.sync.dma_start(
                    out=zxt_d[:, :, g0 : g0 + 32, :].rearrange("cb p g yx -> p cb g yx"),
                    in_=zs[:],
                )

            with (
                tc.tile_pool(name="s1", bufs=1) as s1p,
                tc.tile_pool(name="zs", bufs=2) as zsp,
                tc.tile_pool(name="zp", bufs=3, space="PSUM") as zpp,
            ):
                h0_sb = {d: s1p.tile([128, G, 9], BF16, name=f"h0{d}", tag=f"h0{d}")
                         for d in ("f", "b")}
                h0pad = [s1p.tile([128, GT, 5, 5], BF16, name=f"h0pad{p}", tag=f"h0pad{p}")
                         for p in range(2)]
                nc.gpsimd.memset(h0pad[0][:], 0.0)
                nc.gpsimd.memset(h0pad[1][:], 0.0)

                def c_stage(k, slot, sub):
                    # one quarter of the h0f+h0b pad-add: fine-grained so it
                    # never head-of-line-blocks the recurrence's DVE chain ops
                    ga = k * GT + sub * 32
                    nc.vector.tensor_add(
                        h0pad[slot][:, sub * 32 : sub * 32 + 32, 1:4, 1:4],
                        h0_sb["f"][:, ga : ga + 32, :].rearrange(
                            "p g (y x) -> p g y x", y=3, x=3),
                        h0_sb["b"][:, ga : ga + 32, :].rearrange(
                            "p g (y x) -> p g y x", y=3, x=3),
                    )

                def c_mm_unit(k, slot, sub, d):
                    if d == "f":
                        c_stage(k, slot, sub)
                    hp = h0pad[slot]
                    proj_mm_unit(
                        lambda cb_i, g0, y0, ny, x0, nx2:
                            hp[:, g0 : g0 + 32, y0 : y0 + ny, x0 : x0 + nx2],
                        1,
                        lambda cb_i, tap, cb_o:
                            wx1_sb[d][:, tap, cb_o * 128 : (cb_o + 1) * 128],
                        zx1c[d][k],
                        bias_sb["1" + d],
                        sub * 32,
                    )

                # ======== scope2: x-projection buffers + layer-0 recurrence ====
                with (
                    tc.tile_pool(name="a_w", bufs=1) as awp,
                    tc.tile_pool(name="a_xg", bufs=1) as xgp,
                    tc.tile_pool(name="a_tp", bufs=2, space="PSUM") as tpp,
                    tc.tile_pool(name="l0_st", bufs=1) as stp0,
                    tc.tile_pool(name="l0_zx", bufs=2) as zxp0,
                    tc.tile_pool(name="l0_ps", bufs=1, space="PSUM") as psp0,
                    tc.tile_pool(name="l0_g", bufs=2) as gp0,
                ):
                    wx0_sb = {d: awp.tile([128, 2, 9, 512], BF16, name=f"wx0{d}", tag=f"wx0{d}")
                              for d in ("f", "b")}
                    xpad = [awp.tile([128, 2, GT, 5, 5], BF16, name=f"xpad{p}", tag=f"xpad{p}")
                            for p in range(2)]
                    nc.gpsimd.memset(xpad[0][:], 0.0)
                    nc.gpsimd.memset(xpad[1][:], 0.0)

                    def a_stage(k, slot):
                        t0 = k * chunk_t
                        xg = xgp.tile([GT, C_IN * 9], F32, tag="xg")
                        for ts in range(chunk_t):
                            nc.sync.dma_start(
                                out=xg[ts * BL : (ts + 1) * BL, :],
                                in_=x_gv[t0 + ts],
                            )
                        xgv = xg[:].rearrange("g (c y x) -> g c y x", y=3, x=3)
                        xp = xpad[slot]
                        for cb in range(2):
                            for y in range(3):
                                for xx in range(3):
                                    tp = tpp.tile([128, GT], F32, tag="tp")
                                    nc.tensor.transpose(
                                        tp[:], xgv[:, cb * 128 : (cb + 1) * 128, y, xx],
                                        ident[:GT, :GT],
                                    )
                                    nc.vector.tensor_copy(xp[:, cb, :, 1 + y, 1 + xx], tp[:])

                    def a_mm_unit(k, slot, sub, d):
                        xp = xpad[slot]
                        proj_mm_unit(
                            lambda cb_i, g0, y0, ny, x0, nx2:
                                xp[:, cb_i, g0 : g0 + 32, y0 : y0 + ny, x0 : x0 + nx2],
                            2,
                            lambda cb_i, tap, cb_o:
                                wx0_sb[d][:, cb_i, tap, cb_o * 128 : (cb_o + 1) * 128],
                            zx0c[d][k],
                            bias_sb["0" + d],
                            sub * 32,
                        )

                    rec0 = _Recurrence(nc, stp0, zxp0, psp0, gp0, wh0_sb, zx0c,
                                       h0_sb, identb, "l0", T, chunk_t)

                    # ---- weave: prelude of 2 chunks, rest as B-step filler ----
                    # stage(order[0]) leads the DMA queue with its x loads;
                    # wx0 rides behind them; bulk weights (first needed by
                    # B step 0, ~100us in) come after.
                    order = [0, 7, 1, 6, 2, 5, 3, 4]
                    a_stage(order[0], 0)
                    for d in ("f", "b"):
                        nc.sync.dma_start(out=wx0_sb[d][:], in_=wx0[d][:])
                    a_stage(order[1], 1)
                    for t_sb, t_dram in big_dmas:
                        nc.sync.dma_start(out=t_sb[:], in_=t_dram[:])
                    for i in (0, 1):
                        for sub in range(n_sub):
                            for d in ("f", "b"):
                                a_mm_unit(order[i], i, sub, d)
                    units = []  # (emit_fn, deadline_B_step)
                    for i in range(2, len(order)):
                        k = order[i]
                        slot = i % 2
                        ddl = i // 2 * 16  # chunk pair (2p, 2p+1) needed by B step 16p
                        units.append(((lambda k=k, slot=slot: a_stage(k, slot)), ddl))
                        for sub in range(n_sub):
                            for d in ("f", "b"):
                                units.append((
                                    (lambda k=k, slot=slot, sub=sub, d=d:
                                     a_mm_unit(k, slot, sub, d)),
                                    ddl,
                                ))
                    # C filler: chunk k ready after B step r(k); 3,4 then 2,5
                    # fill the B tail; 1,6 deferred to D; 0,7 emitted post-B
                    c_sched = []
                    ci = 0
                    for k in (3, 4, 2, 5):
                        rdy = max(16 * k + 15, 127 - 16 * k) + 1
                        for sub in range(n_sub):
                            for d in ("f", "b"):
                                c_sched.append((
                                    (lambda k=k, slot=ci % 2, sub=sub, d=d:
                                     c_mm_unit(k, slot, sub, d)),
                                    rdy,
                                ))
                        ci += 1

                    ui = 0
                    csi = 0
                    for s in range(T):
                        rec0.step(s)
                        # hard-flush units due before the next dependent step
                        while ui < len(units) and units[ui][1] <= s + 1:
                            units[ui][0]()
                            ui += 1
                        # opportunistic: ~1 filler unit per step (unit ~ bubble)
                        emitted = 0
                        while ui < len(units) and emitted < 1:
                            units[ui][0]()
                            ui += 1
                            emitted += 1
                        while csi < len(c_sched) and c_sched[csi][1] <= s and emitted < 1:
                            c_sched[csi][0]()
                            csi += 1
                            emitted += 1
                    while csi < len(c_sched):
                        c_sched[csi][0]()
                        csi += 1

                # ======== scope3: layer-1 recurrence + FC head ========
                with (
                    tc.tile_pool(name="h1_pool", bufs=1) as h1p,
                    tc.tile_pool(name="l1_st", bufs=1) as stp1,
                    tc.tile_pool(name="l1_zx", bufs=2) as zxp1,
                    tc.tile_pool(name="l1_ps", bufs=1, space="PSUM") as psp1,
                    tc.tile_pool(name="l1_g", bufs=2) as gp1,
                    tc.tile_pool(name="e_h", bufs=2) as ehp,
                    tc.tile_pool(name="e_ps", bufs=2, space="PSUM") as epp,
                    tc.tile_pool(name="e_o", bufs=2) as eop,
                ):
                    h1_sb = {d: h1p.tile([128, G, 9], BF16, name=f"h1{d}", tag=f"h1{d}")
                             for d in ("f", "b")}

                    def e_unit(gc):
                        ga = gc * GT
                        h1s = ehp.tile([128, GT, 9], BF16, tag="eh1s")
                        nc.vector.tensor_add(
                            h1s[:],
                            h1_sb["f"][:, ga : ga + GT, :],
                            h1_sb["b"][:, ga : ga + GT, :],
                        )
                        ps = epp.tile([NCLS, GT], F32, tag="eps")
                        for yx in range(9):
                            nc.tensor.matmul(
                                ps[:], fcw_sb[:, yx, :], h1s[:, :, yx],
                                start=(yx == 0), stop=(yx == 8),
                            )
                        ot = eop.tile([NCLS, GT], F32, tag="eo")
                        nc.vector.tensor_scalar_add(ot[:], ps[:], fcb_sb[:, 0:1])
                        nc.sync.dma_start(out=out[:, ga : ga + GT], in_=ot[:])

                    rec1 = _Recurrence(nc, stp1, zxp1, psp1, gp1, wh1_sb, zx1c,
                                       h1_sb, identb, "l1", T, chunk_t)

                    # C chunks 0,7 must precede D; 1,6 weave into D steps 0..15
                    for sub in range(n_sub):
                        for d in ("f", "b"):
                            c_mm_unit(0, 0, sub, d)
                    for sub in range(n_sub):
                        for d in ("f", "b"):
                            c_mm_unit(7, 1, sub, d)
                    dunits = []
                    ci = 0
                    for k in (1, 6):
                        slot = ci % 2
                        for sub in range(n_sub):
                            for d in ("f", "b"):
                                dunits.append((
                                    (lambda k=k, slot=slot, sub=sub, d=d:
                                     c_mm_unit(k, slot, sub, d)),
                                    15,
                                ))
                        ci += 1
                    e_sched = sorted(
                        (max(16 * gc + 15, 127 - 16 * gc) + 1, gc) for gc in range(n_gt)
                    )
                    ui = 0
                    ei = 0
                    for s in range(T):
                        rec1.step(s)
                        while ui < len(dunits) and dunits[ui][1] <= s + 1:
                            dunits[ui][0]()
                            ui += 1
                        emitted = 0
                        while ui < len(dunits) and emitted < 1:
                            dunits[ui][0]()
                            ui += 1
                            emitted += 1
                        while ei < len(e_sched) and e_sched[ei][0] <= s:
                            e_unit(e_sched[ei][1])
                            ei += 1
                    while ei < len(e_sched):
                        e_unit(e_sched[ei][1])
                        ei += 1

    _orig_to_json = nc.to_json_bytes
    nc.to_json_bytes = lambda: _fix_multi_waits(_orig_to_json())
    return nc


# ---------------- host side ----------------

def _prep_weights(w, b, cin):
    """w: (512, cin+128, 3, 3) -> (wx, wh) bf16 host arrays + bias (128,4) f32."""
    bf = ml_dtypes.bfloat16
    wx = w[:, :cin].reshape(512, cin, 9)            # (co, ci, tap)
    wx = wx.transpose(1, 2, 0)                      # (ci, tap, co)
    if cin == 256:
        wx = wx.reshape(2, 128, 9, 512).transpose(1, 0, 2, 3)  # (128, 2, 9, 512)
    wx = np.ascontiguousarray(wx).astype(bf)
    wh = w[:, cin:].reshape(512, 128, 9).transpose(1, 2, 0)    # (128, 9, 512)
    wh = np.ascontiguousarray(wh).astype(bf)
    bias = np.ascontiguousarray(b.reshape(4, 128).T).astype(np.float32)
    return wx, wh, bias


def make_inputs_core(core, x, w_f0, b_f0, w_b0, b_b0, w_f1, b_f1, w_b1, b_b1,
                     fc_w, fc_b):
    m = {"x": np.ascontiguousarray(x[core * BL : (core + 1) * BL])}
    for d, w, b in (("f", w_f0, b_f0), ("b", w_b0, b_b0)):
        wx, wh, bias = _prep_weights(np.asarray(w), np.asarray(b), 256)
        m[f"wx0{d}"], m[f"wh0{d}"], m[f"bias0{d}"] = wx, wh, bias
    for d, w, b in (("f", w_f1, b_f1), ("b", w_b1, b_b1)):
        wx, wh, bias = _prep_weights(np.asarray(w), np.asarray(b), 128)
        m[f"wx1{d}"], m[f"wh1{d}"], m[f"bias1{d}"] = wx, wh, bias
    fcw = np.asarray(fc_w).reshape(NCLS, 128, 9).transpose(1, 2, 0)  # (128, 9, 7)
    m["fcw"] = np.ascontiguousarray(fcw).astype(ml_dtypes.bfloat16)
    m["fcb"] = np.ascontiguousarray(np.asarray(fc_b).reshape(NCLS, 1)).astype(np.float32)
    return m


_nc_cache = {}


def kernel(**inputs):
    from concourse.bass_utils import run_bass_kernel_spmd

    if "nc" not in _nc_cache:
        _nc_cache["nc"] = build_program(T_FULL)
    nc = _nc_cache["nc"]
    x = np.asarray(inputs["x"], dtype=np.float32)
    in_maps = [make_inputs_core(c, x, inputs["w_f0"], inputs["b_f0"],
                                inputs["w_b0"], inputs["b_b0"],
                                inputs["w_f1"], inputs["b_f1"],
                                inputs["w_b1"], inputs["b_b1"],
                                inputs["fc_w"], inputs["fc_b"])
               for c in range(NCORES)]
    res = run_bass_kernel_spmd(nc, in_maps, core_ids=list(range(NCORES)))
    outs = []
    for c in range(NCORES):
        o = res.results[c]["out"]  # (7, G) with g = t*BL + b
        o = o.reshape(NCLS, T_FULL, BL).transpose(2, 1, 0)  # (BL, T, 7)
        outs.append(o)
    return np.ascontiguousarray(np.concatenate(outs, axis=0), dtype=np.float32)


# revision 37
# speedup vs baseline: 1.5052x; 1.1719x over previous
"""Bidirectional 2-layer ConvLSTM (3x3 grid) + FC head, Trainium2 Bass kernel.

Sharding: data-parallel over batch. B=64 across 8 cores -> 8 batches/core.
Weights replicated; no inter-core communication.

Single merged instruction stream per core. The recurrence (B/D) leaves
~1.1us PE bubbles per step (gate-chain latency exceeds the other
direction's matmul work), so the feed-forward phases are woven into those
bubbles instead of running as separate phases:

  prelude: x-projection chunks 0,7,1,6 (both time-order heads)
  B steps 0..47   (+) x-projection chunks 2,5,3,4 as filler
  B steps 80..127 (+) layer-1 projection chunks 3,4,2,5 (ready mid-B)
  C chunks 0,7; D steps 0..15 (+) C chunks 1,6; D tail (+) FC chunks

zx scratch lives in DRAM as per-chunk tensors (16 time-steps each) so
dependency tracking stays chunk-granular. zx is bf16 with biases folded
in; it is added into the conv PSUM accumulation via a bf16 identity
matmul, and gates read straight from PSUM. h stays in SBUF (bf16).
"""

import numpy as np
import ml_dtypes

import concourse.bass as bass
import concourse.mybir as mybir
from concourse.tile import TileContext
from concourse.masks import make_identity

BF16 = mybir.dt.bfloat16
F32 = mybir.dt.float32
FP8 = mybir.dt.float8e4
WSCALE = 64.0  # fp8 weight pre-scale: keeps |w*S| ~1, clear of e4m3 subnormals

B_FULL, T_FULL, C_IN, H, NCLS = 64, 128, 256, 128, 7
NCORES = 8
BL = B_FULL // NCORES  # local batch = 8
ZB = 4  # zx DMA time-step batching in the recurrence
CLIPPED = True  # clipped-tap matmuls (multi-dim PSUM out APs, HW-validated)

# taps ordered center-first so the first matmul of each accumulation group
# covers every output column (has_written semantics)
TAPS = [(1, 1)] + [(dy, dx) for dy in range(3) for dx in range(3) if (dy, dx) != (1, 1)]

SIG = mybir.ActivationFunctionType.Sigmoid
TANH = mybir.ActivationFunctionType.Tanh


def _clip(d):
    # output-pixel range [p0, p0+n) and source range [s0, s0+n) for tap offset d
    if d == 0:
        return 1, 0, 2
    if d == 1:
        return 0, 0, 3
    return 0, 1, 2


def _patch_tile_drain():
    """This walrus rejects >1 sync wait on a Drain: keep the first wait on the
    drain and move the rest onto single-wait NOPs executed just before it."""
    from bass_rust import ScopedClock

    if getattr(TileContext, "_drain_patched", False):
        return

    def _drain_and_barrier(self, tick_clock, wait_clock):
        nc = self.nc
        drain_inst = nc.sync.drain()
        wait_clock.add_sem_waits(
            drain_inst.ins, ScopedClock({None: tick_clock.global_clock})
        )
        si = drain_inst.ins.sync_info
        waits = list(si.on_wait)
        if len(waits) > 1:
            while len(si.on_wait) > 1:
                si.on_wait.pop()
            for w in waits[1:]:
                nop = nc.sync.nop()
                nop.ins.sync_info = mybir.SyncInfo(on_wait=[w], on_update=[])
        nc.all_engine_barrier()
        assert self.sems is not None
        popped = nc._tile_sem_poison_stack.pop()
        assert popped is self._sem_poison
        nc.clear_and_free_semaphores(list(self.sems.allocated().values()))
        nc.all_engine_barrier()

    TileContext._drain_and_barrier = _drain_and_barrier
    TileContext._drain_patched = True


def _fix_multi_waits(raw):
    """This walrus accepts at most 1 sync wait per instruction (2 for
    EventSemaphore). Hoist excess waits onto single-wait EventSemaphore
    carriers inserted just before the instruction on the same engine."""
    import json

    d = json.loads(raw)
    nid = 0
    for fn in d["functions"]:
        for blk in fn["blocks"]:
            out = []
            for inst in blk["instructions"]:
                si = inst.get("sync_info")
                ow = (si or {}).get("on_wait") or []
                cap = 2 if inst.get("opcode") == "EventSemaphore" else 1
                if len(ow) > cap:
                    for w in ow[cap:]:
                        nid += 1
                        out.append({
                            "debug": inst.get("debug", 0),
                            "engine": inst["engine"],
                            "ins": [],
                            "name": f"I-xwait-{nid}",
                            "opcode": "EventSemaphore",
                            "outs": [],
                            "sync_info": {"on_update": [], "on_wait": [w]},
                        })
                    si["on_wait"] = ow[:cap]
                out.append(inst)
            blk["instructions"] = out
    return json.dumps(d).encode()


class _Recurrence:
    """One bidirectional ConvLSTM layer, emitted one step at a time so
    feed-forward filler work can be woven between steps."""

    def __init__(self, nc, stp, zxp, psp, gp, wh_sb, zxc, h_sb, identb, name,
                 T, chunk_t):
        self.nc = nc
        self.zxp = zxp
        self.psp = psp
        self.gp = gp
        self.wh_sb = wh_sb
        self.zxc = zxc  # per-chunk DRAM zx tensors, keyed by dir
        self.h_sb = h_sb
        self.identb = identb
        self.name = name
        self.T = T
        self.chunk_t = chunk_t
        self.zxt = {}
        self.hpad = {}
        self.cst = {}
        for d in ("f", "b"):
            # pixel-major (y, x, b) fp8 h: tap-pair windows flatten to 3D
            # DoubleRow moving APs [K, 2, 3*BL]
            self.hpad[d] = [
                stp.tile([128, 5, 5, BL], FP8, name=f"{name}hp{d}{p}", tag=f"{name}hp{d}{p}")
                for p in range(2)
            ]
            nc.gpsimd.memset(self.hpad[d][0][:], 0.0)
            nc.gpsimd.memset(self.hpad[d][1][:], 0.0)
            self.cst[d] = stp.tile([128, BL * 9], F32, name=f"{name}c{d}", tag=f"{name}c{d}")
            nc.gpsimd.memset(self.cst[d][:], 0.0)

    def step(self, s):
        nc = self.nc
        T = self.T
        tt = {"f": s, "b": T - 1 - s}
        zp = {}
        sfio = {}
        tg = {}
        # PE: both dirs' matmuls first, so the engine has a full step of
        # runway while the other dir's gate chain drains
        for d in ("f", "b"):
            t = tt[d]
            hp_r = self.hpad[d][s % 2]
            if s % ZB == 0:
                tz = t if d == "f" else t - (ZB - 1)
                ck = tz // self.chunk_t
                tl = tz % self.chunk_t
                zt = self.zxp.tile([128, 4, ZB, BL * 9], BF16,
                                   name=f"{self.name}zx{d}", tag=f"{self.name}zx{d}")
                nc.sync.dma_start(
                    out=zt[:],
                    in_=self.zxc[d][ck][:, :, tl * BL : (tl + ZB) * BL, :].rearrange(
                        "cb p (zb b) yx -> p cb zb (b yx)", zb=ZB
                    ),
                )
                self.zxt[d] = zt
            zi = s % ZB if d == "f" else ZB - 1 - s % ZB
            zp[d] = self.psp.tile([128, 4, BL * 9], F32,
                                  name=f"{self.name}zp{d}", tag=f"{self.name}zp{d}")
            # g-gate block (cb 3) first: its tanh runs on ACT while PE works
            # through the i/f/o blocks, taking it off the gate critical path.
            # zp columns are pixel-major (y, x, b); the ident matmul reorders
            # the (b, yx)-ordered zx via a strided moving AP. zx carries
            # z*WSCALE (matching the fp8 weight pre-scale); the gate
            # activations divide it back out via their scale argument.
            for cb in (3, 0, 1, 2):
                nc.tensor.matmul(
                    zp[d][:, cb], self.identb[:],
                    self.zxt[d][:, cb, zi].rearrange("p (b yx) -> p yx b", b=BL),
                    start=True, stop=False,
                )
                for oy in range(3):
                    for t1, t2 in ((0, 1), (2, 3), (4, 5), (6, 7)):
                        dy1, dx1 = divmod(t1, 3)
                        dy2, dx2 = divmod(t2, 3)
                        base = hp_r[:, oy + dy1, dx1 : dx1 + 3, :]
                        delta = ((dy2 - dy1) * 5 + (dx2 - dx1)) * BL
                        r_ap = bass.AP(
                            base.tensor, base.offset,
                            [list(base.ap[0]), [delta, 2], [1, 3 * BL]],
                        )
                        nc.tensor.matmul(
                            zp[d][:, cb, oy * 3 * BL : (oy + 1) * 3 * BL],
                            self.wh_sb[d][:, t1 : t1 + 2, cb * 128 : (cb + 1) * 128],
                            r_ap,
                            start=False, stop=False,
                            perf_mode=mybir.MatmulPerfMode.DoubleRow,
                        )
                    nc.tensor.matmul(
                        zp[d][:, cb, oy * 3 * BL : (oy + 1) * 3 * BL],
                        self.wh_sb[d][:, 8, cb * 128 : (cb + 1) * 128],
                        hp_r[:, oy + 2, 2:5, :].rearrange("p x b -> p (x b)"),
                        start=False, stop=(oy == 2),
                    )
                if cb == 3:
                    tg[d] = self.gp.tile([128, BL * 9], F32,
                                         name=f"{self.name}tg{d}", tag=f"{self.name}tg{d}")
                    nc.scalar.activation(tg[d][:], zp[d][:, 3], TANH,
                                         scale=1.0 / WSCALE)
        # gates: one sigmoid over i,f,o; reads straight from PSUM
        for d in ("f", "b"):
            sfio[d] = self.gp.tile([128, 3, BL * 9], F32,
                                   name=f"{self.name}sfio{d}", tag=f"{self.name}sfio{d}")
            nc.scalar.activation(sfio[d][:], zp[d][:, 0:3], SIG, scale=1.0 / WSCALE)
        tcell = {}
        for d in ("f", "b"):
            ig = self.gp.tile([128, BL * 9], F32, name=f"{self.name}ig{d}", tag=f"{self.name}ig{d}")
            cf = self.gp.tile([128, BL * 9], F32, name=f"{self.name}cf{d}", tag=f"{self.name}cf{d}")
            nc.vector.tensor_mul(ig[:], sfio[d][:, 0], tg[d][:])
            nc.vector.tensor_mul(cf[:], sfio[d][:, 1], self.cst[d][:])
            nc.vector.tensor_add(self.cst[d][:], ig[:], cf[:])
            tcell[d] = self.gp.tile([128, BL * 9], F32, name=f"{self.name}tc{d}", tag=f"{self.name}tc{d}")
            nc.scalar.activation(tcell[d][:], self.cst[d][:], TANH)
        for d in ("f", "b"):
            # h = so * tanh(c): straight into the padded tile (critical path
            # to the next step's conv, on DVE) and into the SBUF h buffer on
            # the otherwise-idle GPSIMD engine (off the critical path).
            # Gate tiles are pixel-major (y, x, b); h_sb stays (b, yx).
            hp_w = self.hpad[d][(s + 1) % 2]
            nc.vector.tensor_mul(
                hp_w[:, 1:4, 1:4, :],
                sfio[d][:, 2].rearrange("p (y x b) -> p y x b", y=3, x=3),
                tcell[d][:].rearrange("p (y x b) -> p y x b", y=3, x=3),
            )
            nc.gpsimd.tensor_mul(
                self.h_sb[d][:, tt[d] * BL : (tt[d] + 1) * BL, :].rearrange(
                    "p b (y x) -> p y x b", y=3, x=3),
                sfio[d][:, 2].rearrange("p (y x b) -> p y x b", y=3, x=3),
                tcell[d][:].rearrange("p (y x b) -> p y x b", y=3, x=3),
            )


def build_program(T=T_FULL, phases="ABCDE"):
    """Build the per-core Bass program (merged stream). Returns nc."""
    _patch_tile_drain()
    G = T * BL
    GT = 128 if G % 128 == 0 else G  # groups per chunk
    assert G % GT == 0 and GT % 32 == 0
    n_gt = G // GT
    n_sub = GT // 32
    chunk_t = GT // BL  # time-steps per chunk (16)
    assert n_gt == 8 and chunk_t % ZB == 0

    nc = bass.Bass()

    # ---- I/O ----
    x = nc.dram_tensor("x", [BL, T, C_IN, 3, 3], F32, kind="ExternalInput")
    wx0 = {}
    wh0 = {}
    wx1 = {}
    wh1 = {}
    bias_in = {}
    for d in ("f", "b"):
        wx0[d] = nc.dram_tensor(f"wx0{d}", [128, 2, 9, 512], BF16, kind="ExternalInput")
        wh0[d] = nc.dram_tensor(f"wh0{d}", [128, 9, 512], FP8, kind="ExternalInput")
        wx1[d] = nc.dram_tensor(f"wx1{d}", [128, 9, 512], BF16, kind="ExternalInput")
        wh1[d] = nc.dram_tensor(f"wh1{d}", [128, 9, 512], FP8, kind="ExternalInput")
        bias_in[f"0{d}"] = nc.dram_tensor(f"bias0{d}", [128, 4], F32, kind="ExternalInput")
        bias_in[f"1{d}"] = nc.dram_tensor(f"bias1{d}", [128, 4], F32, kind="ExternalInput")
    fcw = nc.dram_tensor("fcw", [128, 9, NCLS], BF16, kind="ExternalInput")
    fcb = nc.dram_tensor("fcb", [NCLS, 1], F32, kind="ExternalInput")
    out = nc.dram_tensor("out", [NCLS, G], F32, kind="ExternalOutput")

    # ---- DRAM scratch: per-chunk zx tensors (bf16, biases folded in) ----
    zx0c = {d: [nc.dram_tensor(f"zx0{d}{k}", [4, 128, GT, 9], BF16) for k in range(n_gt)]
            for d in ("f", "b")}
    zx1c = {d: [nc.dram_tensor(f"zx1{d}{k}", [4, 128, GT, 9], BF16) for k in range(n_gt)]
            for d in ("f", "b")}

    with TileContext(nc) as tc:
        with tc.tile_pool(name="persist", bufs=1) as pp:
            wh0_sb = {d: pp.tile([128, 9, 512], FP8, name=f"wh0{d}", tag=f"wh0{d}") for d in ("f", "b")}
            wx1_sb = {d: pp.tile([128, 9, 512], BF16, name=f"wx1{d}", tag=f"wx1{d}") for d in ("f", "b")}
            wh1_sb = {d: pp.tile([128, 9, 512], FP8, name=f"wh1{d}", tag=f"wh1{d}") for d in ("f", "b")}
            bias_sb = {}
            big_dmas = []  # bulk weight loads, issued after the first x loads
            for d in ("f", "b"):
                big_dmas += [(wh0_sb[d], wh0[d]), (wx1_sb[d], wx1[d]),
                             (wh1_sb[d], wh1[d])]
                for l in ("0", "1"):
                    bias_sb[l + d] = pp.tile([128, 4], F32, name=f"bias{l}{d}", tag=f"bias{l}{d}")
                    nc.sync.dma_start(out=bias_sb[l + d][:], in_=bias_in[l + d][:])
            fcw_sb = pp.tile([128, 9, NCLS], BF16, tag="fcw")
            big_dmas.append((fcw_sb, fcw))
            fcb_sb = pp.tile([NCLS, 1], F32, tag="fcb")
            nc.sync.dma_start(out=fcb_sb[:], in_=fcb[:])
            ident = pp.tile([128, 128], F32, tag="ident")
            make_identity(nc, ident[:])
            identb = pp.tile([128, 128], BF16, tag="identb")
            make_identity(nc, identb[:])
            # biases pre-scaled by WSCALE, for the ACT-side zs op
            # (out = Identity(zp*WSCALE + bias*WSCALE))
            bias64_sb = {}
            for key, bt in bias_sb.items():
                bias64_sb[key] = pp.tile([128, 4], F32, name=f"b64{key}", tag=f"b64{key}")
                nc.vector.tensor_scalar_mul(bias64_sb[key][:], bt[:], WSCALE)

            x_gv = x[:].rearrange("b t c y x -> t b (c y x)")  # (T, BL, 2304)

            def proj_mm_unit(xp_ap, n_cbi, w_of, zxt_d, bias_key, g0):
                """One (sub, dir) projection unit: 4 gate blocks x taps
                matmuls + bias-fold copies (bf16) + one DMA store."""
                zs = zsp.tile([128, 4, 32, 9], BF16, tag="zs")
                for cb_o in range(4):
                    zp = zpp.tile([128, 32, 3, 3], F32, tag="zp")
                    zpf = zp[:].rearrange("p g y x -> p (g y x)")
                    k = 0
                    for dy, dx in TAPS:
                        py, sy, ny = _clip(dy)
                        px, sx, nx2 = _clip(dx)
                        clipped = CLIPPED and not (ny == 3 and nx2 == 3)
                        for cb_i in range(n_cbi):
                            w_ap = w_of(cb_i, dy * 3 + dx, cb_o)
                            if clipped:
                                o_ap = zp[:, :, py : py + ny, px : px + nx2]
                                r_ap = xp_ap(cb_i, g0, 1 + sy, ny, 1 + sx, nx2)
                            else:
                                o_ap = zpf
                                r_ap = xp_ap(cb_i, g0, dy, 3, dx, 3)
                            nc.tensor.matmul(
                                o_ap, w_ap, r_ap,
                                start=(k == 0), stop=(k == 9 * n_cbi - 1),
                            )
                            k += 1
                    # zx stored as (z + bias) * WSCALE so it accumulates on the
                    # same scale as the fp8 recurrent-weight matmul products.
                    # Split across ACT and DVE so neither engine's queue blocks
                    # the recurrence's gate ops for long.
                    zp_view = zp[:].rearrange("p g y x -> p g (y x)")
                    if cb_o < 2:
                        nc.scalar.activation(
                            zs[:, cb_o], zp_view,
                            mybir.ActivationFunctionType.Identity,
                            bias=bias64_sb[bias_key][:, cb_o : cb_o + 1],
                            scale=WSCALE,
                        )
                    else:
                        nc.vector.tensor_scalar(
                            zs[:, cb_o], zp_view,
                            bias_sb[bias_key][:, cb_o : cb_o + 1], WSCALE,
                            mybir.AluOpType.add, mybir.AluOpType.mult,
                        )
                nc.sync.dma_start(
                    out=zxt_d[:, :, g0 : g0 + 32, :].rearrange("cb p g yx -> p cb g yx"),
                    in_=zs[:],
                )

            with (
                tc.tile_pool(name="s1", bufs=1) as s1p,
                tc.tile_pool(name="zs", bufs=2) as zsp,
                tc.tile_pool(name="zp", bufs=3, space="PSUM") as zpp,
            ):
                h0_sb = {d: s1p.tile([128, G, 9], BF16, name=f"h0{d}", tag=f"h0{d}")
                         for d in ("f", "b")}
                h0pad = [s1p.tile([128, GT, 5, 5], BF16, name=f"h0pad{p}", tag=f"h0pad{p}")
                         for p in range(2)]
                nc.gpsimd.memset(h0pad[0][:], 0.0)
                nc.gpsimd.memset(h0pad[1][:], 0.0)

                def c_stage(k, slot, sub):
                    # one quarter of the h0f+h0b pad-add: fine-grained so it
                    # never head-of-line-blocks the recurrence's DVE chain ops
                    ga = k * GT + sub * 32
                    nc.vector.tensor_add(
                        h0pad[slot][:, sub * 32 : sub * 32 + 32, 1:4, 1:4],
                        h0_sb["f"][:, ga : ga + 32, :].rearrange(
                            "p g (y x) -> p g y x", y=3, x=3),
                        h0_sb["b"][:, ga : ga + 32, :].rearrange(
                            "p g (y x) -> p g y x", y=3, x=3),
                    )

                def c_mm_unit(k, slot, sub, d):
                    if d == "f":
                        c_stage(k, slot, sub)
                    hp = h0pad[slot]
                    proj_mm_unit(
                        lambda cb_i, g0, y0, ny, x0, nx2:
                            hp[:, g0 : g0 + 32, y0 : y0 + ny, x0 : x0 + nx2],
                        1,
                        lambda cb_i, tap, cb_o:
                            wx1_sb[d][:, tap, cb_o * 128 : (cb_o + 1) * 128],
                        zx1c[d][k],
                        "1" + d,
                        sub * 32,
                    )

                # ======== scope2: x-projection buffers + layer-0 recurrence ====
                with (
                    tc.tile_pool(name="a_w", bufs=1) as awp,
                    tc.tile_pool(name="a_xg", bufs=1) as xgp,
                    tc.tile_pool(name="a_tp", bufs=2, space="PSUM") as tpp,
                    tc.tile_pool(name="l0_st", bufs=1) as stp0,
                    tc.tile_pool(name="l0_zx", bufs=2) as zxp0,
                    tc.tile_pool(name="l0_ps", bufs=1, space="PSUM") as psp0,
                    tc.tile_pool(name="l0_g", bufs=2) as gp0,
                ):
                    wx0_sb = {d: awp.tile([128, 2, 9, 512], BF16, name=f"wx0{d}", tag=f"wx0{d}")
                              for d in ("f", "b")}
                    xpad = [awp.tile([128, 2, GT, 5, 5], BF16, name=f"xpad{p}", tag=f"xpad{p}")
                            for p in range(2)]
                    nc.gpsimd.memset(xpad[0][:], 0.0)
                    nc.gpsimd.memset(xpad[1][:], 0.0)

                    def a_stage(k, slot):
                        t0 = k * chunk_t
                        xg = xgp.tile([GT, C_IN * 9], F32, tag="xg")
                        for ts in range(chunk_t):
                            nc.sync.dma_start(
                                out=xg[ts * BL : (ts + 1) * BL, :],
                                in_=x_gv[t0 + ts],
                            )
                        xgv = xg[:].rearrange("g (c y x) -> g c y x", y=3, x=3)
                        xp = xpad[slot]
                        for cb in range(2):
                            for y in range(3):
                                for xx in range(3):
                                    tp = tpp.tile([128, GT], F32, tag="tp")
                                    nc.tensor.transpose(
                                        tp[:], xgv[:, cb * 128 : (cb + 1) * 128, y, xx],
                                        ident[:GT, :GT],
                                    )
                                    nc.vector.tensor_copy(xp[:, cb, :, 1 + y, 1 + xx], tp[:])

                    def a_mm_unit(k, slot, sub, d):
                        xp = xpad[slot]
                        proj_mm_unit(
                            lambda cb_i, g0, y0, ny, x0, nx2:
                                xp[:, cb_i, g0 : g0 + 32, y0 : y0 + ny, x0 : x0 + nx2],
                            2,
                            lambda cb_i, tap, cb_o:
                                wx0_sb[d][:, cb_i, tap, cb_o * 128 : (cb_o + 1) * 128],
                            zx0c[d][k],
                            "0" + d,
                            sub * 32,
                        )

                    rec0 = _Recurrence(nc, stp0, zxp0, psp0, gp0, wh0_sb, zx0c,
                                       h0_sb, identb, "l0", T, chunk_t)

                    # ---- weave: prelude of 2 chunks, rest as B-step filler ----
                    # stage(order[0]) leads the DMA queue with its x loads;
                    # wx0 rides behind them; bulk weights (first needed by
                    # B step 0, ~100us in) come after.
                    order = [0, 7, 1, 6, 2, 5, 3, 4]
                    a_stage(order[0], 0)
                    for d in ("f", "b"):
                        nc.sync.dma_start(out=wx0_sb[d][:], in_=wx0[d][:])
                    a_stage(order[1], 1)
                    for t_sb, t_dram in big_dmas:
                        nc.sync.dma_start(out=t_sb[:], in_=t_dram[:])
                    # stage(i+1) is interleaved halfway through chunk i's mm
                    # units: its DVE copies overlap chunk i's matmuls instead
                    # of stalling chunk i+1's. (Slot conflict is only with
                    # chunk i-1, which is fully emitted by then.)
                    def mm_units_of(i):
                        return [(order[i], i % 2, sub, d)
                                for sub in range(n_sub) for d in ("f", "b")]

                    for j, (k, slot, sub, d) in enumerate(mm_units_of(0)):
                        a_mm_unit(k, slot, sub, d)
                        if j == 3:
                            a_stage(order[2], 0)
                    for j, (k, slot, sub, d) in enumerate(mm_units_of(1)):
                        a_mm_unit(k, slot, sub, d)
                        if j == 3:
                            a_stage(order[3], 1)
                    units = []  # (emit_fn, deadline_B_step)
                    for i in range(2, len(order)):
                        ddl = i // 2 * 16  # chunk pair (2p, 2p+1) needed by B step 16p
                        for j, (k, slot, sub, d) in enumerate(mm_units_of(i)):
                            units.append((
                                (lambda k=k, slot=slot, sub=sub, d=d:
                                 a_mm_unit(k, slot, sub, d)),
                                ddl,
                            ))
                            if j == 3 and i + 2 < len(order):
                                ns_ = order[i + 2]
                                units.append((
                                    (lambda ns_=ns_, sl=(i + 2) % 2: a_stage(ns_, sl)),
                                    ddl,
                                ))
                    # C filler: chunk k ready after B step r(k); 3,4 then 2,5
                    # fill the B tail; 1,6 deferred to D; 0,7 emitted post-B
                    c_sched = []
                    ci = 0
                    for k in (3, 4, 2, 5):
                        rdy = max(16 * k + 15, 127 - 16 * k) + 1
                        for sub in range(n_sub):
                            for d in ("f", "b"):
                                c_sched.append((
                                    (lambda k=k, slot=ci % 2, sub=sub, d=d:
                                     c_mm_unit(k, slot, sub, d)),
                                    rdy,
                                ))
                        ci += 1

                    ui = 0
                    csi = 0
                    for s in range(T):
                        rec0.step(s)
                        # hard-flush units due before the next dependent step
                        while ui < len(units) and units[ui][1] <= s + 1:
                            units[ui][0]()
                            ui += 1
                        # opportunistic: ~1 filler unit per step (unit ~ bubble)
                        emitted = 0
                        while ui < len(units) and emitted < 1:
                            units[ui][0]()
                            ui += 1
                            emitted += 1
                        while csi < len(c_sched) and c_sched[csi][1] <= s and emitted < 1:
                            c_sched[csi][0]()
                            csi += 1
                            emitted += 1
                    while csi < len(c_sched):
                        c_sched[csi][0]()
                        csi += 1

                # ======== scope3: layer-1 recurrence + FC head ========
                with (
                    tc.tile_pool(name="h1_pool", bufs=1) as h1p,
                    tc.tile_pool(name="l1_st", bufs=1) as stp1,
                    tc.tile_pool(name="l1_zx", bufs=2) as zxp1,
                    tc.tile_pool(name="l1_ps", bufs=1, space="PSUM") as psp1,
                    tc.tile_pool(name="l1_g", bufs=2) as gp1,
                    tc.tile_pool(name="e_h", bufs=2) as ehp,
                    tc.tile_pool(name="e_ps", bufs=2, space="PSUM") as epp,
                    tc.tile_pool(name="e_o", bufs=2) as eop,
                ):
                    h1_sb = {d: h1p.tile([128, G, 9], BF16, name=f"h1{d}", tag=f"h1{d}")
                             for d in ("f", "b")}

                    def e_unit(gc):
                        ga = gc * GT
                        h1s = ehp.tile([128, GT, 9], BF16, tag="eh1s")
                        nc.vector.tensor_add(
                            h1s[:],
                            h1_sb["f"][:, ga : ga + GT, :],
                            h1_sb["b"][:, ga : ga + GT, :],
                        )
                        ps = epp.tile([NCLS, GT], F32, tag="eps")
                        for yx in range(9):
                            nc.tensor.matmul(
                                ps[:], fcw_sb[:, yx, :], h1s[:, :, yx],
                                start=(yx == 0), stop=(yx == 8),
                            )
                        ot = eop.tile([NCLS, GT], F32, tag="eo")
                        nc.vector.tensor_scalar_add(ot[:], ps[:], fcb_sb[:, 0:1])
                        nc.sync.dma_start(out=out[:, ga : ga + GT], in_=ot[:])

                    rec1 = _Recurrence(nc, stp1, zxp1, psp1, gp1, wh1_sb, zx1c,
                                       h1_sb, identb, "l1", T, chunk_t)

                    # C chunks 0,7 must precede D; 1,6 weave into D steps 0..15
                    for sub in range(n_sub):
                        for d in ("f", "b"):
                            c_mm_unit(0, 0, sub, d)
                    for sub in range(n_sub):
                        for d in ("f", "b"):
                            c_mm_unit(7, 1, sub, d)
                    dunits = []
                    ci = 0
                    for k in (1, 6):
                        slot = ci % 2
                        for sub in range(n_sub):
                            for d in ("f", "b"):
                                dunits.append((
                                    (lambda k=k, slot=slot, sub=sub, d=d:
                                     c_mm_unit(k, slot, sub, d)),
                                    15,
                                ))
                        ci += 1
                    e_sched = sorted(
                        (max(16 * gc + 15, 127 - 16 * gc) + 1, gc) for gc in range(n_gt)
                    )
                    ui = 0
                    ei = 0
                    for s in range(T):
                        rec1.step(s)
                        while ui < len(dunits) and dunits[ui][1] <= s + 1:
                            dunits[ui][0]()
                            ui += 1
                        emitted = 0
                        while ui < len(dunits) and emitted < 1:
                            dunits[ui][0]()
                            ui += 1
                            emitted += 1
                        while ei < len(e_sched) and e_sched[ei][0] <= s:
                            e_unit(e_sched[ei][1])
                            ei += 1
                    while ei < len(e_sched):
                        e_unit(e_sched[ei][1])
                        ei += 1

    _orig_to_json = nc.to_json_bytes
    nc.to_json_bytes = lambda: _fix_multi_waits(_orig_to_json())
    return nc


# ---------------- host side ----------------

def _prep_weights(w, b, cin):
    """w: (512, cin+128, 3, 3) -> (wx, wh) bf16 host arrays + bias (128,4) f32."""
    bf = ml_dtypes.bfloat16
    wx = w[:, :cin].reshape(512, cin, 9)            # (co, ci, tap)
    wx = wx.transpose(1, 2, 0)                      # (ci, tap, co)
    if cin == 256:
        wx = wx.reshape(2, 128, 9, 512).transpose(1, 0, 2, 3)  # (128, 2, 9, 512)
    wx = np.ascontiguousarray(wx).astype(bf)
    wh = w[:, cin:].reshape(512, 128, 9).transpose(1, 2, 0)    # (128, 9, 512)
    # fp8, pre-scaled: |wh*WSCALE| ~ 1 sits in e4m3's sweet spot
    wh = np.ascontiguousarray(wh * WSCALE).astype(ml_dtypes.float8_e4m3)
    bias = np.ascontiguousarray(b.reshape(4, 128).T).astype(np.float32)
    return wx, wh, bias


def make_inputs_core(core, x, w_f0, b_f0, w_b0, b_b0, w_f1, b_f1, w_b1, b_b1,
                     fc_w, fc_b):
    m = {"x": np.ascontiguousarray(x[core * BL : (core + 1) * BL])}
    for d, w, b in (("f", w_f0, b_f0), ("b", w_b0, b_b0)):
        wx, wh, bias = _prep_weights(np.asarray(w), np.asarray(b), 256)
        m[f"wx0{d}"], m[f"wh0{d}"], m[f"bias0{d}"] = wx, wh, bias
    for d, w, b in (("f", w_f1, b_f1), ("b", w_b1, b_b1)):
        wx, wh, bias = _prep_weights(np.asarray(w), np.asarray(b), 128)
        m[f"wx1{d}"], m[f"wh1{d}"], m[f"bias1{d}"] = wx, wh, bias
    fcw = np.asarray(fc_w).reshape(NCLS, 128, 9).transpose(1, 2, 0)  # (128, 9, 7)
    m["fcw"] = np.ascontiguousarray(fcw).astype(ml_dtypes.bfloat16)
    m["fcb"] = np.ascontiguousarray(np.asarray(fc_b).reshape(NCLS, 1)).astype(np.float32)
    return m


_nc_cache = {}


def kernel(**inputs):
    from concourse.bass_utils import run_bass_kernel_spmd

    if "nc" not in _nc_cache:
        _nc_cache["nc"] = build_program(T_FULL)
    nc = _nc_cache["nc"]
    x = np.asarray(inputs["x"], dtype=np.float32)
    in_maps = [make_inputs_core(c, x, inputs["w_f0"], inputs["b_f0"],
                                inputs["w_b0"], inputs["b_b0"],
                                inputs["w_f1"], inputs["b_f1"],
                                inputs["w_b1"], inputs["b_b1"],
                                inputs["fc_w"], inputs["fc_b"])
               for c in range(NCORES)]
    res = run_bass_kernel_spmd(nc, in_maps, core_ids=list(range(NCORES)))
    outs = []
    for c in range(NCORES):
        o = res.results[c]["out"]  # (7, G) with g = t*BL + b
        o = o.reshape(NCLS, T_FULL, BL).transpose(2, 1, 0)  # (BL, T, 7)
        outs.append(o)
    return np.ascontiguousarray(np.concatenate(outs, axis=0), dtype=np.float32)
